# revision 37
# baseline (speedup 1.0000x reference)
"""BERT input representation kernel for 8 TRN2 NeuronCores.

Math (reference):
    x1  = x @ W_emb + b_emb                      # [B,S,D]
    seg = einsum('bnsd,s->bnd', x1.reshape(B,S/8,8,D), w_seg) + b_seg
    out = (x1.reshape(...) + seg[:,:,None,:]).reshape(B,S,D) + PE(S,D)

Folded form used here (exact algebra):
    out[b,s,:] = (A @ x[b])[s,:] @ W_emb + bias[s,:]
where A = I + blockdiag(ones(8,1) @ w_seg[None,:]) mixes rows within each
8-row segment, and bias[s,:] = PE[s,:] + b_emb*(1 + sum(w_seg)) + b_seg.

Sharding: pure data-parallel over batch; each of 8 cores handles 8
batches (4096 rows = 32 row-tiles of 128 rows = 16 tile-pair groups).

Schedule (v2) — the kernel is paced by the 16 MiB/core f32 output
store stream and, secondarily, by the PSUM->SBUF drain work:

  - output DRAM layout is partition-major ([128, 32*1024]; host
    un-transposes), so a store of G consecutive row-tiles is one DMA
    with a single contiguous G*4KiB chunk per partition — large
    descriptors at near-peak HBM write rate. Store group sizes
    [1,1,2,4,8,8,8] keep the stream dense from ~4 us onward, all on
    the sync HWDGE ring (which carries nothing else).
  - all input loads are issued up-front on the scalar ring into
    persistent buffers (no pool recycling -> no semaphore waits that
    would block the ring FIFO), ordered by first use.
  - PSUM drain work is split across two engines: even tile-pair
    groups drain via DVE tensor_add (bias add + drain in one op),
    odd groups pre-fill the PSUM quarter with the bias via a PE
    identity-matmul, accumulate the main matmul on top (start=False),
    and drain with a 1-input ACT copy. This halves the per-engine
    drain time that paced the v1 kernel.
  - per wave: PE builds x~^T for a pair of row-tiles per matmul
    (psum[128,128] = [x_i|x_j].T @ A^T = transpose + segment-mix in
    one op), ACT copies it to a resident bf16 tile; mains are K=64
    matmuls (u=0 on PE rows 0-63, u=1 on rows 64-127 via a duplicated
    W copy).
"""

import sys

if "/opt/trn_rl_repo" not in sys.path:
    sys.path.insert(0, "/opt/trn_rl_repo")

import ml_dtypes
import numpy as np

import concourse.bacc as bacc
import concourse.mybir as mybir
import concourse.tile as tile
from concourse.bass_utils import run_bass_kernel_spmd

B, S, F, D, SEG = 64, 512, 64, 1024, 8
N_CORES = 8
B_LOC = B // N_CORES          # batches per core
ROWS = B_LOC * S              # 4096 rows per core
TILE_P = 128                  # rows per tile
N_TILES = ROWS // TILE_P      # 32
N_PAIR = N_TILES // 2         # 16 tile-pairs
N_BIAS = S // TILE_P          # 4 distinct bias row-tiles
HD = D // 2                   # 512

# combined-constants column layout (bf16): [W2 | I | bias0..bias3]
CC_W = 0                      # cols [0:1024] = W stacked twice
CC_I = D                      # cols [1024:1152] = 128x128 identity
CC_B = D + TILE_P             # cols [1152:5248] = 4 bias row-tiles
CC_COLS = CC_B + N_BIAS * D   # 5248

_NC_CACHE = None


def _build_nc():
    nc = bacc.Bacc("TRN2", target_bir_lowering=False, debug=False,
                   num_devices=N_CORES)
    # x pre-rearranged on host (layout + cast to the kernel's bf16
    # compute precision): xr[p, i*F:(i+1)*F] = x[i*128+p]
    # cols [0:128] = A^T, then the rearranged x
    x_d = nc.declare_dram_parameter("x", [TILE_P, TILE_P + N_TILES * F],
                                    mybir.dt.bfloat16, isOutput=False)
    cc_d = nc.declare_dram_parameter("cc", [TILE_P, CC_COLS],
                                     mybir.dt.bfloat16, isOutput=False)
    # partition-major output: out_d[p, i*D:(i+1)*D] = out row i*128+p
    out_d = nc.declare_dram_parameter("out", [TILE_P, N_TILES * D],
                                      mybir.dt.float32, isOutput=True)

    with tile.TileContext(nc) as tc:
        with (
            tc.tile_pool(name="const", bufs=1) as cpool,
            tc.tile_pool(name="ps_t", bufs=1, space="PSUM") as pst,
            tc.tile_pool(name="ps_o", bufs=7, space="PSUM") as pso,
        ):
            # persistent SBUF buffers — one tile per DMA so dependency
            # tracking never serializes a consumer on an unrelated load
            L0C = TILE_P + 4 * F          # A^T + first 2 pairs of x
            x0_sb = cpool.tile([TILE_P, L0C], mybir.dt.bfloat16)
            xr_sb = cpool.tile([TILE_P, N_TILES * F - 4 * F],
                               mybir.dt.bfloat16)          # x pairs 2-15
            w_sb = cpool.tile([TILE_P, CC_B], mybir.dt.bfloat16)
            b01_sb = cpool.tile([TILE_P, 2 * D], mybir.dt.bfloat16)
            b23_sb = cpool.tile([TILE_P, 2 * D], mybir.dt.bfloat16)
            xt_sb = cpool.tile([TILE_P, N_PAIR * TILE_P],
                               mybir.dt.bfloat16)          # x~^T resident
            obuf = cpool.tile([TILE_P, N_TILES * D], mybir.dt.float32)

            at_ap = x0_sb[:, 0:TILE_P]

            def bias_ap(jb, lo, hi):
                sb = b01_sb if jb < 2 else b23_sb
                col = (jb % 2) * D
                return sb[:, col + lo:col + hi]

            def w_ap(u, lo, hi):
                return w_sb[64 * u:64 * u + F, lo:hi]

            def xp_ap(k):
                # x columns of pair k (128 cols)
                if k < 2:
                    return x0_sb[:, TILE_P + 128 * k:TILE_P + 128 * (k + 1)]
                return xr_sb[:, 128 * (k - 2):128 * (k - 1)]

            i_ap = w_sb[:, CC_I:CC_I + TILE_P]

            # ---- loads: critical head on the sync ring (clears before
            # the first store is ready), bulk on the scalar ring (which
            # can't start until ~8.6 us anyway due to ACT_TABLE_LOAD in
            # the preamble, but runs concurrently with early stores).
            # Each ring's SECOND DMA pays ~2-4us of completion-sem
            # latency behind the first's receipt chain, so the two loads
            # that gate the first PE ops go FIRST on separate rings:
            # W alone heads the sync ring, x0 heads the scalar ring.
            nc.sync.dma_start(w_sb[:], cc_d[:, 0:CC_B])
            nc.scalar.dma_start(x0_sb[:], x_d[:, 0:L0C])
            # bias tiles 0,1 (gate the first DVE adds)
            nc.scalar.dma_start(b01_sb[:], cc_d[:, CC_B:CC_B + 2 * D])
            # bias tiles 2,3 (gate the first ACT-group bias-matmuls)
            nc.scalar.dma_start(b23_sb[:], cc_d[:, CC_B + 2 * D:])
            # rest of x
            nc.scalar.dma_start(xr_sb[:], x_d[:, L0C:])

            # ---- waves of tile-pairs: transposes (phase 1) then
            # matmul+drain (phase 2); store groups ship as they complete.
            WAVES = [2, 2, 4, 4, 4]
            pr0 = 0
            for wn, wp in enumerate(WAVES):
                c0, cw = pr0 * TILE_P, wp * TILE_P
                ps_x = pst.tile([TILE_P, 512], mybir.dt.float32,
                                name="ps_x", tag="ps_x")
                for k in range(wp):
                    nc.tensor.matmul(ps_x[:, 128 * k:128 * (k + 1)],
                                     xp_ap(pr0 + k),
                                     at_ap, start=True, stop=True)
                nc.scalar.copy(xt_sb[:, c0:c0 + cw], ps_x[:, 0:cw])

                for j in range(pr0, pr0 + wp):
                    # four single-bank PSUM quarters per group: [u][half]
                    q = [[pso.tile([TILE_P, HD], mybir.dt.float32,
                                   name=f"q{u}{h}", tag="q")
                          for h in range(2)] for u in range(2)]
                    jbs = ((2 * j) % N_BIAS, (2 * j + 1) % N_BIAS)
                    lhss = tuple(
                        xt_sb[64 * u:64 * (u + 1), 128 * j:128 * (j + 1)]
                        for u in range(2))
                    # drain engine per group: DVE (13 groups) / ACT (3);
                    # balanced so PE (mains + ACT-group bias-matmuls,
                    # ~34us effective) and DVE adds (~624ns/half) finish
                    # together. ACT groups sit at j%4==2 so their
                    # bias-matmuls read bias tiles 0,1 — the first bias
                    # load to land; j=14 stays DVE to trim PE's share.
                    act_grp = (j % 4 == 2 and j != 14)
                    if act_grp:
                        # pre-fill quarters with bias via identity-matmul
                        for u in range(2):
                            for h in range(2):
                                nc.tensor.matmul(
                                    q[u][h][:], i_ap,
                                    bias_ap(jbs[u], h * HD, (h + 1) * HD),
                                    start=True, stop=False)
                    # mains; u=0 uses PE rows 0-63, u=1 rows 64-127
                    for u in range(2):
                        nc.tensor.matmul(q[u][0][:], lhss[u],
                                         w_ap(u, 0, HD),
                                         start=not act_grp, stop=True)
                    for u in range(2):
                        nc.tensor.matmul(q[u][1][:], lhss[u],
                                         w_ap(u, HD, D),
                                         start=not act_grp, stop=True)
                    # drain: ACT groups plain copy (bias already in PSUM),
                    # DVE groups tensor_add (bias+drain in one op)
                    for u in range(2):
                        i = 2 * j + u
                        oc = i * D
                        if act_grp:
                            nc.scalar.copy(obuf[:, oc:oc + HD], q[u][0][:])
                            nc.scalar.copy(obuf[:, oc + HD:oc + D],
                                           q[u][1][:])
                        else:
                            nc.vector.tensor_add(obuf[:, oc:oc + HD],
                                                 q[u][0][:],
                                                 bias_ap(jbs[u], 0, HD))
                            nc.vector.tensor_add(obuf[:, oc + HD:oc + D],
                                                 q[u][1][:],
                                                 bias_ap(jbs[u], HD, D))
                    # ship this group's 2 tiles as one 1 MiB store. The
                    # sync ring otherwise lags the last drain by ~12us
                    # (serialized ~0.6us receipt per store), so the last
                    # three drain-gated stores fan out across the OTHER
                    # rings — j13,j15 on scalar (ACT idle by then), j14
                    # on the gpsimd SWDGE ring — so no tail store queues
                    # behind more than one other. (Tried and rejected:
                    # rotating EVERY other store onto a second queue —
                    # the SDMA packet round-robin drops both rings' HBM
                    # rate 427->370 GB/s; 2-4 MiB merged stores gate on
                    # late drains; tail-only overlap is cheap.)
                    lo, hi = 2 * j * D, (2 * j + 2) * D
                    if j == N_PAIR - 2:
                        eng = nc.gpsimd
                    elif j >= N_PAIR - 3:
                        eng = nc.scalar
                    else:
                        eng = nc.sync
                    eng.dma_start(out_d[:, lo:hi], obuf[:, lo:hi])
                pr0 += wp
    nc.compile()
    return nc


def _host_constants(W_emb, b_emb, w_seg, b_seg):
    # sinusoidal positional encoding, float32, same formula as the reference
    pos = np.arange(S, dtype=np.float32)[:, None]
    div = np.exp(np.arange(0, D, 2, dtype=np.float32)
                 * (-np.log(10000.0) / D)).astype(np.float32)
    ang = pos * div
    pe = np.zeros((S, D), np.float32)
    pe[:, 0::2] = np.sin(ang)
    pe[:, 1::2] = np.cos(ang)

    bias = (pe + b_emb[None, :] * (np.float32(1.0) + w_seg.sum())
            + b_seg[0]).astype(np.float32)
    # rearrange to [128, 4*D]: column block j holds bias rows j*128..j*128+127
    bias_r = np.ascontiguousarray(
        bias.reshape(N_BIAS, TILE_P, D).transpose(1, 0, 2).reshape(
            TILE_P, N_BIAS * D)).astype(ml_dtypes.bfloat16)

    blk = np.eye(SEG, dtype=np.float32) + w_seg[:, None] * np.ones(
        (1, SEG), np.float32)
    at = np.kron(np.eye(TILE_P // SEG, dtype=np.float32), blk).astype(
        ml_dtypes.bfloat16)

    wb = np.vstack([W_emb, W_emb]).astype(ml_dtypes.bfloat16)
    ident = np.eye(TILE_P, dtype=np.float32).astype(ml_dtypes.bfloat16)
    # combined consts: [W2 | I | bias0..bias3] as [128, CC_COLS] bf16
    cc = np.ascontiguousarray(np.concatenate([wb, ident, bias_r], axis=1))
    return at, cc


def _prepare_in_maps(x, W_emb, b_emb, w_seg, b_seg):
    x = np.ascontiguousarray(np.asarray(x, dtype=np.float32))
    W_emb = np.asarray(W_emb, dtype=np.float32)
    b_emb = np.asarray(b_emb, dtype=np.float32)
    w_seg = np.asarray(w_seg, dtype=np.float32)
    b_seg = np.asarray(b_seg, dtype=np.float32)

    at, cc = _host_constants(W_emb, b_emb, w_seg, b_seg)

    in_maps = []
    for c in range(N_CORES):
        xs = x[c * B_LOC:(c + 1) * B_LOC].reshape(ROWS, F)
        # rearrange [32 tiles, 128 rows, F] -> [128, 32*F], bf16 staging
        xr = np.ascontiguousarray(
            xs.reshape(N_TILES, TILE_P, F).transpose(1, 0, 2).reshape(
                TILE_P, N_TILES * F)).astype(ml_dtypes.bfloat16)
        in_maps.append(
            {"x": np.ascontiguousarray(np.concatenate([at, xr], axis=1)),
             "cc": cc})
    return in_maps


def kernel(x, W_emb, b_emb, w_seg, b_seg):
    in_maps = _prepare_in_maps(x, W_emb, b_emb, w_seg, b_seg)

    global _NC_CACHE
    if _NC_CACHE is None:
        _NC_CACHE = _build_nc()

    res = run_bass_kernel_spmd(_NC_CACHE, in_maps,
                               core_ids=list(range(N_CORES)))
    # un-transpose the partition-major output: [128, 32*D] -> [B_LOC, S, D]
    out = np.concatenate(
        [np.asarray(res.results[c]["out"])
         .reshape(TILE_P, N_TILES, D).transpose(1, 0, 2)
         .reshape(B_LOC, S, D)
         for c in range(N_CORES)], axis=0)
    return out


# revision 38
# speedup vs baseline: 1.0726x; 1.0726x over previous
"""BERT input representation kernel for 8 TRN2 NeuronCores.

Math (reference):
    x1  = x @ W_emb + b_emb                      # [B,S,D]
    seg = einsum('bnsd,s->bnd', x1.reshape(B,S/8,8,D), w_seg) + b_seg
    out = (x1.reshape(...) + seg[:,:,None,:]).reshape(B,S,D) + PE(S,D)

Folded form used here (exact algebra):
    out[b,s,:] = (A @ x[b])[s,:] @ W_emb + bias[s,:]
where A = I + blockdiag(ones(8,1) @ w_seg[None,:]) mixes rows within each
8-row segment, and bias[s,:] = PE[s,:] + b_emb*(1 + sum(w_seg)) + b_seg.

Sharding: pure data-parallel over batch; each of 8 cores handles 8
batches (4096 rows = 32 row-tiles of 128 rows = 16 tile-pair groups).

Schedule (v2) — the kernel is paced by the 16 MiB/core f32 output
store stream and, secondarily, by the PSUM->SBUF drain work:

  - output DRAM layout is partition-major ([128, 32*1024]; host
    un-transposes), so a store of G consecutive row-tiles is one DMA
    with a single contiguous G*4KiB chunk per partition — large
    descriptors at near-peak HBM write rate. Store group sizes
    [1,1,2,4,8,8,8] keep the stream dense from ~4 us onward, all on
    the sync HWDGE ring (which carries nothing else).
  - all input loads are issued up-front on the scalar ring into
    persistent buffers (no pool recycling -> no semaphore waits that
    would block the ring FIFO), ordered by first use.
  - PSUM drain work is split across two engines: even tile-pair
    groups drain via DVE tensor_add (bias add + drain in one op),
    odd groups pre-fill the PSUM quarter with the bias via a PE
    identity-matmul, accumulate the main matmul on top (start=False),
    and drain with a 1-input ACT copy. This halves the per-engine
    drain time that paced the v1 kernel.
  - per wave: PE builds x~^T for a pair of row-tiles per matmul
    (psum[128,128] = [x_i|x_j].T @ A^T = transpose + segment-mix in
    one op), ACT copies it to a resident bf16 tile; mains are K=64
    matmuls (u=0 on PE rows 0-63, u=1 on rows 64-127 via a duplicated
    W copy).
"""

import sys

if "/opt/trn_rl_repo" not in sys.path:
    sys.path.insert(0, "/opt/trn_rl_repo")

import ml_dtypes
import numpy as np

import concourse.bacc as bacc
import concourse.mybir as mybir
import concourse.tile as tile
from concourse.bass_utils import run_bass_kernel_spmd

B, S, F, D, SEG = 64, 512, 64, 1024, 8
N_CORES = 8
B_LOC = B // N_CORES          # batches per core
ROWS = B_LOC * S              # 4096 rows per core
TILE_P = 128                  # rows per tile
N_TILES = ROWS // TILE_P      # 32
N_PAIR = N_TILES // 2         # 16 tile-pairs
N_BIAS = S // TILE_P          # 4 distinct bias row-tiles
HD = D // 2                   # 512

# combined-constants column layout (bf16): [W2 | I | bias0..bias3]
CC_W = 0                      # cols [0:1024] = W stacked twice
CC_I = D                      # cols [1024:1152] = 128x128 identity
CC_B = D + TILE_P             # cols [1152:5248] = 4 bias row-tiles
CC_COLS = CC_B + N_BIAS * D   # 5248

_NC_CACHE = None


def _build_nc():
    nc = bacc.Bacc("TRN2", target_bir_lowering=False, debug=False,
                   num_devices=N_CORES)
    # x pre-rearranged on host (layout + cast to the kernel's bf16
    # compute precision): xr[p, i*F:(i+1)*F] = x[i*128+p]
    # cols [0:128] = A^T, then the rearranged x
    x_d = nc.declare_dram_parameter("x", [TILE_P, TILE_P + N_TILES * F],
                                    mybir.dt.bfloat16, isOutput=False)
    cc_d = nc.declare_dram_parameter("cc", [TILE_P, CC_COLS],
                                     mybir.dt.bfloat16, isOutput=False)
    # partition-major output: out_d[p, i*D:(i+1)*D] = out row i*128+p
    out_d = nc.declare_dram_parameter("out", [TILE_P, N_TILES * D],
                                      mybir.dt.float32, isOutput=True)

    with tile.TileContext(nc) as tc:
        with (
            tc.tile_pool(name="const", bufs=1) as cpool,
            tc.tile_pool(name="ps_t", bufs=1, space="PSUM") as pst,
            tc.tile_pool(name="ps_o", bufs=7, space="PSUM") as pso,
        ):
            # persistent SBUF buffers — one tile per DMA so dependency
            # tracking never serializes a consumer on an unrelated load
            L0C = TILE_P + 4 * F          # A^T + first 2 pairs of x
            x0_sb = cpool.tile([TILE_P, L0C], mybir.dt.bfloat16)
            xr_sb = cpool.tile([TILE_P, N_TILES * F - 4 * F],
                               mybir.dt.bfloat16)          # x pairs 2-15
            w_sb = cpool.tile([TILE_P, CC_B], mybir.dt.bfloat16)
            b01_sb = cpool.tile([TILE_P, 2 * D], mybir.dt.bfloat16)
            b23_sb = cpool.tile([TILE_P, 2 * D], mybir.dt.bfloat16)
            xt_sb = cpool.tile([TILE_P, N_PAIR * TILE_P],
                               mybir.dt.bfloat16)          # x~^T resident
            obuf = cpool.tile([TILE_P, N_TILES * D], mybir.dt.float32)

            at_ap = x0_sb[:, 0:TILE_P]

            def bias_ap(jb, lo, hi):
                sb = b01_sb if jb < 2 else b23_sb
                col = (jb % 2) * D
                return sb[:, col + lo:col + hi]

            def w_ap(u, lo, hi):
                return w_sb[64 * u:64 * u + F, lo:hi]

            def xp_ap(k):
                # x columns of pair k (128 cols)
                if k < 2:
                    return x0_sb[:, TILE_P + 128 * k:TILE_P + 128 * (k + 1)]
                return xr_sb[:, 128 * (k - 2):128 * (k - 1)]

            i_ap = w_sb[:, CC_I:CC_I + TILE_P]

            # ---- loads: critical head on the sync ring (clears before
            # the first store is ready), bulk on the scalar ring (which
            # can't start until ~8.6 us anyway due to ACT_TABLE_LOAD in
            # the preamble, but runs concurrently with early stores).
            # Each ring's SECOND DMA pays ~2-4us of completion-sem
            # latency behind the first's receipt chain, so the two loads
            # that gate the first PE ops go FIRST on separate rings:
            # W alone heads the sync ring, x0 heads the scalar ring.
            nc.sync.dma_start(w_sb[:], cc_d[:, 0:CC_B])
            nc.scalar.dma_start(x0_sb[:], x_d[:, 0:L0C])
            # bias tiles 0,1 (gate the first DVE adds)
            nc.scalar.dma_start(b01_sb[:], cc_d[:, CC_B:CC_B + 2 * D])
            # bias tiles 2,3 (gate the first ACT-group bias-matmuls)
            nc.scalar.dma_start(b23_sb[:], cc_d[:, CC_B + 2 * D:])
            # rest of x
            nc.scalar.dma_start(xr_sb[:], x_d[:, L0C:])

            # ---- waves of tile-pairs: transposes (phase 1) then
            # matmul+drain (phase 2); store groups ship as they complete.
            WAVES = [2, 2, 4, 4, 4]
            pr0 = 0
            for wn, wp in enumerate(WAVES):
                c0, cw = pr0 * TILE_P, wp * TILE_P
                ps_x = pst.tile([TILE_P, 512], mybir.dt.float32,
                                name="ps_x", tag="ps_x")
                for k in range(wp):
                    nc.tensor.matmul(ps_x[:, 128 * k:128 * (k + 1)],
                                     xp_ap(pr0 + k),
                                     at_ap, start=True, stop=True)
                nc.scalar.copy(xt_sb[:, c0:c0 + cw], ps_x[:, 0:cw])

                for j in range(pr0, pr0 + wp):
                    # four single-bank PSUM quarters per group: [u][half]
                    q = [[pso.tile([TILE_P, HD], mybir.dt.float32,
                                   name=f"q{u}{h}", tag="q")
                          for h in range(2)] for u in range(2)]
                    jbs = ((2 * j) % N_BIAS, (2 * j + 1) % N_BIAS)
                    lhss = tuple(
                        xt_sb[64 * u:64 * (u + 1), 128 * j:128 * (j + 1)]
                        for u in range(2))
                    # drain engine per group: DVE (13 groups) / ACT (3);
                    # balanced so PE (mains + ACT-group bias-matmuls,
                    # ~34us effective) and DVE adds (~624ns/half) finish
                    # together. ACT groups sit at j%4==2 so their
                    # bias-matmuls read bias tiles 0,1 — the first bias
                    # load to land; j=14 stays DVE to trim PE's share.
                    act_grp = (j % 4 == 2 and j != 14)
                    if act_grp:
                        # pre-fill quarters with bias via identity-matmul
                        for u in range(2):
                            for h in range(2):
                                nc.tensor.matmul(
                                    q[u][h][:], i_ap,
                                    bias_ap(jbs[u], h * HD, (h + 1) * HD),
                                    start=True, stop=False)
                    # mains; u=0 uses PE rows 0-63, u=1 rows 64-127
                    for u in range(2):
                        nc.tensor.matmul(q[u][0][:], lhss[u],
                                         w_ap(u, 0, HD),
                                         start=not act_grp, stop=True)
                    for u in range(2):
                        nc.tensor.matmul(q[u][1][:], lhss[u],
                                         w_ap(u, HD, D),
                                         start=not act_grp, stop=True)
                    # drain: ACT groups plain copy (bias already in PSUM),
                    # DVE groups tensor_add (bias+drain in one op)
                    for u in range(2):
                        i = 2 * j + u
                        oc = i * D
                        if act_grp:
                            nc.scalar.copy(obuf[:, oc:oc + HD], q[u][0][:])
                            nc.scalar.copy(obuf[:, oc + HD:oc + D],
                                           q[u][1][:])
                        else:
                            nc.vector.tensor_add(obuf[:, oc:oc + HD],
                                                 q[u][0][:],
                                                 bias_ap(jbs[u], 0, HD))
                            nc.vector.tensor_add(obuf[:, oc + HD:oc + D],
                                                 q[u][1][:],
                                                 bias_ap(jbs[u], HD, D))
                    # ship this group's 2 tiles as one 1 MiB store. The
                    # sync ring otherwise lags the last drain by ~12us
                    # (serialized ~0.6us receipt per store), so the last
                    # three groups go on the scalar ring — ACT has no
                    # compute left by then, and those stores are
                    # drain-gated regardless, so the rings' tails
                    # overlap. (Tried and rejected: rotating EVERY other
                    # store onto a second queue — the SDMA packet
                    # round-robin drops both rings' HBM rate 427->370
                    # GB/s; ANY gpsimd/SWDGE store adds an expensive
                    # GpSimd dge_drain to the exit barrier, ~+4.5us;
                    # 2-4 MiB merged stores gate on late drains.)
                    lo, hi = 2 * j * D, (2 * j + 2) * D
                    eng = nc.scalar if j >= N_PAIR - 3 else nc.sync
                    eng.dma_start(out_d[:, lo:hi], obuf[:, lo:hi])
                pr0 += wp
    nc.compile()
    return nc


def _host_constants(W_emb, b_emb, w_seg, b_seg):
    # sinusoidal positional encoding, float32, same formula as the reference
    pos = np.arange(S, dtype=np.float32)[:, None]
    div = np.exp(np.arange(0, D, 2, dtype=np.float32)
                 * (-np.log(10000.0) / D)).astype(np.float32)
    ang = pos * div
    pe = np.zeros((S, D), np.float32)
    pe[:, 0::2] = np.sin(ang)
    pe[:, 1::2] = np.cos(ang)

    bias = (pe + b_emb[None, :] * (np.float32(1.0) + w_seg.sum())
            + b_seg[0]).astype(np.float32)
    # rearrange to [128, 4*D]: column block j holds bias rows j*128..j*128+127
    bias_r = np.ascontiguousarray(
        bias.reshape(N_BIAS, TILE_P, D).transpose(1, 0, 2).reshape(
            TILE_P, N_BIAS * D)).astype(ml_dtypes.bfloat16)

    blk = np.eye(SEG, dtype=np.float32) + w_seg[:, None] * np.ones(
        (1, SEG), np.float32)
    at = np.kron(np.eye(TILE_P // SEG, dtype=np.float32), blk).astype(
        ml_dtypes.bfloat16)

    wb = np.vstack([W_emb, W_emb]).astype(ml_dtypes.bfloat16)
    ident = np.eye(TILE_P, dtype=np.float32).astype(ml_dtypes.bfloat16)
    # combined consts: [W2 | I | bias0..bias3] as [128, CC_COLS] bf16
    cc = np.ascontiguousarray(np.concatenate([wb, ident, bias_r], axis=1))
    return at, cc


def _prepare_in_maps(x, W_emb, b_emb, w_seg, b_seg):
    x = np.ascontiguousarray(np.asarray(x, dtype=np.float32))
    W_emb = np.asarray(W_emb, dtype=np.float32)
    b_emb = np.asarray(b_emb, dtype=np.float32)
    w_seg = np.asarray(w_seg, dtype=np.float32)
    b_seg = np.asarray(b_seg, dtype=np.float32)

    at, cc = _host_constants(W_emb, b_emb, w_seg, b_seg)

    in_maps = []
    for c in range(N_CORES):
        xs = x[c * B_LOC:(c + 1) * B_LOC].reshape(ROWS, F)
        # rearrange [32 tiles, 128 rows, F] -> [128, 32*F], bf16 staging
        xr = np.ascontiguousarray(
            xs.reshape(N_TILES, TILE_P, F).transpose(1, 0, 2).reshape(
                TILE_P, N_TILES * F)).astype(ml_dtypes.bfloat16)
        in_maps.append(
            {"x": np.ascontiguousarray(np.concatenate([at, xr], axis=1)),
             "cc": cc})
    return in_maps


def kernel(x, W_emb, b_emb, w_seg, b_seg):
    in_maps = _prepare_in_maps(x, W_emb, b_emb, w_seg, b_seg)

    global _NC_CACHE
    if _NC_CACHE is None:
        _NC_CACHE = _build_nc()

    res = run_bass_kernel_spmd(_NC_CACHE, in_maps,
                               core_ids=list(range(N_CORES)))
    # un-transpose the partition-major output: [128, 32*D] -> [B_LOC, S, D]
    out = np.concatenate(
        [np.asarray(res.results[c]["out"])
         .reshape(TILE_P, N_TILES, D).transpose(1, 0, 2)
         .reshape(B_LOC, S, D)
         for c in range(N_CORES)], axis=0)
    return out
